# revision 6
# baseline (speedup 1.0000x reference)
"""GCN (3x GCNConv + global max pool + MLP) on 8 Trainium2 NeuronCores.

Strategy (v3 — node-major transforms + tile-major tables + delayed RS):
  - Nodes blocked by graph ownership: core c owns contiguous local rows
    [0, N_PAD); global padded row = c*N_PAD + r.
  - Edges are assigned to the core owning their SRC node, so every gather
    reads the core-LOCAL bounce table (int16 indices, single table).
  - Each layer: gather src rows (bf16) -> one-hot S matmuls (bf16)
    accumulate [F, WIN]-window partials over the GLOBAL dst space ->
    partial table P[c_blk, F, N_PAD] fp8 -> ReduceScatter(add) gives each
    core its fully-reduced agg [F, N_PAD].  RS for half 0 is ISSUED a few
    gather-calls after its windows complete so the Pool sequencer (which
    owns both SWDGE desc-gen and the collective dispatch) never stalls
    desc-gen behind the collective's input semaphores.
  - Self-loops are the post-RS diagonal term (dinv^2 * h_prev), added
    feature-major into the transform slab.
  - Transform + next-layer bounce table in ONE pass: for each 128-node
    tile, matmul(lhsT=slab[:,tile], rhs=[W;b]) yields a node-major psum
    tile [128, F_out]; one Activation op applies relu, scales by the
    per-node dinv (scale-ptr), converts to bf16 and lands directly in the
    staged write buffer.  No transpose pass, no separate rescale.
  - Bounce tables are TILE-MAJOR (row of node v = (v%128)*NTIL + v//128)
    so staged writes are >=1.2KB-contiguous per partition (no 2x small-
    descriptor DMA penalty); gather indices are host-remapped to match.
  - L1 pre-transforms x@W1 on-device (z = (dinv*x)@W1 commutes with agg);
    its post-agg "transform" is the identity+bias matmul [I;b1].
  - Pooling: slot-gather (48 slots/graph) from tile-major h3t with 768B
    rows + transpose + batched reduce_max.  MLP feature-major bf16.
"""
import os
import sys

sys.path.insert(0, "/opt/trn_rl_repo")

import numpy as np

import concourse.bass as bass
import concourse.mybir as mybir
import concourse.tile as tile
from concourse import bacc
from concourse.bass_utils import run_bass_kernel_spmd
from concourse.masks import make_identity

F32 = mybir.dt.float32
BF16 = mybir.dt.bfloat16
FP8 = mybir.dt.float8e4
I16 = mybir.dt.int16
NC = 8
WIN = 256          # max dst-window width (S-matmul free size)
SLOT = 48          # pooling slots per graph (max graph = 46)
GBLK = 8           # blocks per dma_gather call (1024-idx HW ring limit)
CHUNK = 512        # feature-major column chunk (psum-bank bound)
TGRP = 5           # node-tiles per staged table-write group
RS_DELAY = 6       # gather calls between half-0 window completion and RS issue

try:
    import ml_dtypes
    _BF = ml_dtypes.bfloat16
except ImportError:  # pragma: no cover
    _BF = np.float32


def _ceil(a, b):
    return -(-a // b)


def _bf(x):
    return np.asarray(x, np.float32).astype(_BF)


def _wrap_idx(flat):
    # dma_gather index layout: idx i -> partition i%16, col i//16, replicated x8
    w = flat.reshape(-1, 16).T.astype(np.int16)
    return np.tile(w, (8, 1))


def _plan(x, edge_index, batch, weights):
    N, XD = x.shape
    G = 2048 if N == 50000 else int(batch.max()) + 1
    assert G % NC == 0
    GPC = G // NC

    batch = np.asarray(batch, dtype=np.int64)
    sizes = np.bincount(batch, minlength=G)
    assert sizes.min() >= 1 and sizes.max() <= SLOT
    gcore = np.arange(G) // GPC
    node_core = gcore[batch]

    core_start = np.searchsorted(batch, np.arange(NC) * GPC)
    core_start = np.concatenate([core_start, [N]])
    ncounts = np.diff(core_start)
    N_PAD = _ceil(int(ncounts.max()), WIN) * WIN
    assert N_PAD % 256 == 0 and N_PAD < 32768
    NTIL = N_PAD // 128

    local_row = np.arange(N) - core_start[node_core]
    g_row = node_core * N_PAD + local_row

    # tile-major remap: node local row r -> table row (r%128)*NTIL + r//128
    def remap(r):
        return (r % 128) * NTIL + r // 128

    src = np.asarray(edge_index[0], dtype=np.int64)
    dst = np.asarray(edge_index[1], dtype=np.int64)
    deg = (np.bincount(dst, minlength=N) + 1).astype(np.float32)
    dinv = (1.0 / np.sqrt(deg)).astype(np.float32)

    # real edges only; self-loops become the post-RS diagonal add
    e_core = node_core[src]
    e_idx = remap(local_row[src]).astype(np.int16)
    e_sval = dinv[dst]

    # variable-width windows: greedy boundaries per (half, core-block)
    # region so each window's max-core edge count just fills K_T blocks
    # (stream order = all half-0 regions first, so RS half 0 fires mid-layer)
    HCOL = N_PAD // 2
    gdst = g_row[dst]
    cnt_cr = np.zeros((NC, NC * N_PAD), np.int32)
    for c in range(NC):
        cnt_cr[c] = np.bincount(gdst[e_core == c], minlength=NC * N_PAD)
    CAP_E, CAP_W = 128 * 3, WIN
    wstart, wwid, wcb, wh, Klist = [], [], [], [], []
    warr = np.zeros(NC * N_PAD, np.int64)
    for h in range(2):
        for cb in range(NC):
            r0 = cb * N_PAD + h * HCOL
            run = np.zeros(NC, np.int64)
            w0 = r0
            for r in range(r0, r0 + HCOL):
                nxt = run + cnt_cr[:, r]
                if r > w0 and (nxt.max() > CAP_E or r - w0 >= CAP_W):
                    wstart.append(w0); wwid.append(r - w0)
                    wcb.append(cb); wh.append(h)
                    Klist.append(max(1, _ceil(int(run.max()), 128)))
                    warr[w0:r] = len(wstart) - 1
                    w0 = r
                    run = cnt_cr[:, r].astype(np.int64)
                else:
                    run = nxt
            wstart.append(w0); wwid.append(r0 + HCOL - w0)
            wcb.append(cb); wh.append(h)
            Klist.append(max(1, _ceil(int(run.max()), 128)))
            warr[w0:r0 + HCOL] = len(wstart) - 1
    NWG = len(wstart)
    K = np.asarray(Klist, np.int64)
    wstart = np.asarray(wstart, np.int64)
    wwid = np.asarray(wwid, np.int64)
    assert wwid.max() <= WIN
    NBLK = int(K.sum())
    E_cap = NBLK * 128
    e_w = warr[gdst]
    e_rel = (gdst - wstart[e_w]).astype(np.float32)
    assert e_rel.min() >= 0 and e_rel.max() < WIN
    key = e_core * NWG + e_w
    blk_off = np.concatenate([[0], np.cumsum(K)[:-1]])

    order = np.lexsort((e_w, e_core))
    s_key = key[order]
    s_idx = e_idx[order]
    s_rel = e_rel[order]
    s_sval = e_sval[order]
    grp_starts = np.searchsorted(s_key, np.arange(NC * NWG))
    grp_ends = np.concatenate([grp_starts[1:], [len(s_key)]])

    idx_w, rel_cols, sval_cols = [], [], []
    for c in range(NC):
        idx16_s = np.zeros(E_cap, np.int16)
        rel_s = np.full(E_cap, -1.0, np.float32)
        sval_s = np.zeros(E_cap, np.float32)
        a = grp_starts[c * NWG:(c + 1) * NWG]
        b = grp_ends[c * NWG:(c + 1) * NWG]
        for w in range(NWG):
            m = b[w] - a[w]
            if m == 0:
                continue
            d0 = blk_off[w] * 128
            idx16_s[d0:d0 + m] = s_idx[a[w]:b[w]]
            rel_s[d0:d0 + m] = s_rel[a[w]:b[w]]
            sval_s[d0:d0 + m] = s_sval[a[w]:b[w]]
        idx_w.append(_wrap_idx(idx16_s))
        rel_cols.append(np.ascontiguousarray(rel_s.reshape(NBLK, 128).T))
        sval_cols.append(np.ascontiguousarray(sval_s.reshape(NBLK, 128).T))

    # pooling slot plan: node local row r lives at h3t row remap(r)+1; row 0 zero
    NSLOT = GPC * SLOT
    gstart = np.concatenate([[0], np.cumsum(sizes)])
    slot_w = []
    for c in range(NC):
        sl = np.zeros(NSLOT, np.int64)
        for j in range(GPC):
            gi = c * GPC + j
            st0 = gstart[gi] - core_start[c]
            sz = sizes[gi]
            sl[j * SLOT:j * SLOT + sz] = remap(np.arange(st0, st0 + sz)) + 1
        slot_w.append(_wrap_idx(sl.astype(np.int16)))

    # per-core node data
    xs = (dinv[:, None] * np.asarray(x, np.float32)).astype(np.float32)
    X1T, d1bc, d2bc, dcol = [], [], [], []
    for c in range(NC):
        n0, n1 = core_start[c], core_start[c + 1]
        xt = np.zeros((XD, N_PAD), np.float32)
        xt[:, : n1 - n0] = xs[n0:n1].T
        X1T.append(_bf(xt))
        dl = np.zeros(N_PAD, np.float32)
        dl[: n1 - n0] = dinv[n0:n1]
        d1bc.append(_bf(np.broadcast_to(dl[None, :], (128, N_PAD))))
        d2bc.append(_bf(np.broadcast_to((dl * dl)[None, :], (128, N_PAD))))
        dcol.append(np.ascontiguousarray(dl.reshape(NTIL, 128).T))

    W1, b1 = weights["W1"], weights["b1"]
    W2, b2 = weights["W2"], weights["b2"]
    W3, b3 = weights["W3"], weights["b3"]
    F1, F2, F3 = W1.shape[1], W2.shape[1], W3.shape[1]
    assert (XD, F1, F2, F3) == (78, 78, 156, 312)
    w_shared = dict(
        W1=_bf(W1),
        W1e=_bf(np.vstack([np.eye(F1, dtype=np.float32),
                           np.asarray(b1, np.float32)[None, :]])),
        b1c=_bf(np.asarray(b1, np.float32).reshape(-1, 1)),
        W2e=_bf(np.vstack([np.asarray(W2, np.float32),
                           np.asarray(b2, np.float32)[None, :]])),
        W3a=_bf(np.asarray(W3[:128], np.float32)),
        W3b=_bf(np.vstack([np.asarray(W3[128:], np.float32),
                           np.asarray(b3, np.float32)[None, :]])),
        Wg1=_bf(weights["Wg1"]), Wg2=_bf(weights["Wg2"]),
        Wf1=_bf(weights["Wf1"]), Wf2=_bf(weights["Wf2"]),
        Wo=_bf(weights["Wo"]),
        bg1=_bf(np.asarray(weights["bg1"], np.float32).reshape(-1, 128).T),
        bg2=np.asarray(weights["bg2"], np.float32).reshape(-1, 1),
        bf1=_bf(np.asarray(weights["bf1"], np.float32).reshape(-1, 128).T),
        bf2=_bf(np.asarray(weights["bf2"], np.float32).reshape(-1, 128).T),
        bo=_bf(np.asarray(weights["bo"], np.float32).reshape(1, 1)),
        iota=_bf(np.broadcast_to(
            np.arange(WIN, dtype=np.float32)[None, :], (128, WIN))),
    )

    meta = dict(
        N=N, XD=XD, G=G, GPC=GPC, N_PAD=N_PAD, NWG=NWG, NTIL=NTIL,
        K=K, NBLK=NBLK, E_cap=E_cap, NSLOT=NSLOT,
        F1=F1, F2=F2, F3=F3,
        D1=weights["Wg1"].shape[1], D2=weights["Wg2"].shape[1],
        D3=weights["Wf1"].shape[1], D4=weights["Wf2"].shape[1],
        WSTART=wstart, WWID=wwid, WCB=np.asarray(wcb), WH=np.asarray(wh),
    )
    in_maps = [
        dict(
            X1T=X1T[c], d1bc=d1bc[c], d2bc=d2bc[c], dcol=dcol[c],
            idx_w=idx_w[c], rel_c=rel_cols[c], sval_c=sval_cols[c],
            slot_w=slot_w[c], **w_shared,
        )
        for c in range(NC)
    ]
    return meta, in_maps


def _build(meta):
    N_PAD, NWG, NTIL = meta["N_PAD"], meta["NWG"], meta["NTIL"]
    K, NBLK, E_cap, NSLOT = meta["K"], meta["NBLK"], meta["E_cap"], meta["NSLOT"]
    GPC, XD = meta["GPC"], meta["XD"]
    F1, F2, F3 = meta["F1"], meta["F2"], meta["F3"]
    D1, D2, D3, D4 = meta["D1"], meta["D2"], meta["D3"], meta["D4"]
    E1, E2 = 128, 256            # bounce row elems (bf16): 256B / 512B
    EP = 384                     # h3 row elems (312 -> 384, 768B)
    HCOL = N_PAD // 2
    HTIL = NTIL // 2
    TG = TGRP if HTIL % TGRP == 0 else 1
    assert GPC * SLOT == NSLOT and NSLOT % 128 == 0

    SL = int(os.environ.get("KRS_STOP", "10"))
    nc = bacc.Bacc("TRN2", target_bir_lowering=False, debug=False,
                   num_devices=NC)

    # ---- I/O ----
    X1T_in = nc.dram_tensor("X1T", [XD, N_PAD], BF16, kind="ExternalInput")
    d1bc_in = nc.dram_tensor("d1bc", [128, N_PAD], BF16, kind="ExternalInput")
    d2bc_in = nc.dram_tensor("d2bc", [128, N_PAD], BF16, kind="ExternalInput")
    dcol_in = nc.dram_tensor("dcol", [128, NTIL], F32, kind="ExternalInput")
    idx_in = nc.dram_tensor("idx_w", [128, E_cap // 16], I16, kind="ExternalInput")
    rel_in = nc.dram_tensor("rel_c", [128, NBLK], F32, kind="ExternalInput")
    sval_in = nc.dram_tensor("sval_c", [128, NBLK], F32, kind="ExternalInput")
    slot_in = nc.dram_tensor("slot_w", [128, NSLOT // 16], I16, kind="ExternalInput")
    iota_in = nc.dram_tensor("iota", [128, WIN], BF16, kind="ExternalInput")
    W1_in = nc.dram_tensor("W1", [XD, F1], BF16, kind="ExternalInput")
    W1e_in = nc.dram_tensor("W1e", [F1 + 1, F1], BF16, kind="ExternalInput")
    b1_in = nc.dram_tensor("b1c", [F1, 1], BF16, kind="ExternalInput")
    W2e_in = nc.dram_tensor("W2e", [F1 + 1, F2], BF16, kind="ExternalInput")
    W3a_in = nc.dram_tensor("W3a", [128, F3], BF16, kind="ExternalInput")
    W3b_in = nc.dram_tensor("W3b", [F2 - 128 + 1, F3], BF16, kind="ExternalInput")
    Wg1_in = nc.dram_tensor("Wg1", [F3, D1], BF16, kind="ExternalInput")
    Wg2_in = nc.dram_tensor("Wg2", [D1, D2], BF16, kind="ExternalInput")
    Wf1_in = nc.dram_tensor("Wf1", [D2, D3], BF16, kind="ExternalInput")
    Wf2_in = nc.dram_tensor("Wf2", [D3, D4], BF16, kind="ExternalInput")
    Wo_in = nc.dram_tensor("Wo", [D4, 1], BF16, kind="ExternalInput")
    bg1_in = nc.dram_tensor("bg1", [128, D1 // 128], BF16, kind="ExternalInput")
    bg2_in = nc.dram_tensor("bg2", [128, 1], F32, kind="ExternalInput")
    bf1_in = nc.dram_tensor("bf1", [128, D3 // 128], BF16, kind="ExternalInput")
    bf2_in = nc.dram_tensor("bf2", [128, D4 // 128], BF16, kind="ExternalInput")
    bo_in = nc.dram_tensor("bo", [1, 1], BF16, kind="ExternalInput")
    out_d = nc.dram_tensor("out_d", [1, GPC], F32, kind="ExternalOutput")

    REL = mybir.ActivationFunctionType.Relu
    CPY = mybir.ActivationFunctionType.Copy
    SIG = mybir.ActivationFunctionType.Sigmoid
    EQ, MUL, ADD, MAX = (
        mybir.AluOpType.is_equal, mybir.AluOpType.mult,
        mybir.AluOpType.add, mybir.AluOpType.max,
    )

    with tile.TileContext(nc) as tc:
        with (
            tc.tile_pool(name="dramp", bufs=1, space="DRAM") as dramp,
            tc.tile_pool(name="const", bufs=1) as constp,
            tc.tile_pool(name="hold", bufs=1) as holdp,
            tc.tile_pool(name="chain", bufs=5) as chainp,
            tc.tile_pool(name="dbc", bufs=1) as dbcp,
        ):
            # DRAM scratch (bounce tables are tile-major: node local row r
            # lives at table row (r%128)*NTIL + r//128)
            B0 = dramp.tile([128 * NTIL, E1], BF16, tag="B0")
            B1 = dramp.tile([128 * NTIL, E1], BF16, tag="B1")
            B2 = dramp.tile([128 * NTIL, E2], BF16, tag="B2")
            P1h = [dramp.tile([NC, F1, HCOL], FP8, name=f"P1{h}",
                              tag=f"P1{h}") for h in range(2)]
            P2h = [dramp.tile([NC, F1, HCOL], FP8, name=f"P2{h}",
                              tag=f"P2{h}") for h in range(2)]
            P3h = [dramp.tile([NC, F2, HCOL], FP8, name=f"P3{h}",
                              tag=f"P3{h}") for h in range(2)]
            A1h = [dramp.tile([F1, HCOL], FP8, name=f"A1{h}",
                              tag=f"A1{h}") for h in range(2)]
            A2h = [dramp.tile([F1, HCOL], FP8, name=f"A2{h}",
                              tag=f"A2{h}") for h in range(2)]
            A3h = [dramp.tile([F2, HCOL], FP8, name=f"A3{h}",
                              tag=f"A3{h}") for h in range(2)]
            h3t = dramp.tile([1 + 128 * NTIL, EP], BF16, tag="h3t")

            def tview(T, elem):
                return T.rearrange("(p t) e -> p t e", t=NTIL)

            h3m = h3t[1:].rearrange("(p t) e -> p t e", t=NTIL)

            # persistent SBUF (edge streams loaded after stage-0 kickoff)
            idx_sb = holdp.tile([128, E_cap // 16], I16)
            rel_sb = holdp.tile([128, NBLK], F32)
            sval_sb = holdp.tile([128, NBLK], F32)
            slot_sb = holdp.tile([128, NSLOT // 16], I16)
            iota_sb = constp.tile([128, WIN], BF16)
            nc.sync.dma_start(iota_sb[:], iota_in[:, :])
            dcol_sb = constp.tile([128, NTIL], F32)
            nc.sync.dma_start(dcol_sb[:], dcol_in[:, :])
            ident = constp.tile([128, 128], BF16)
            make_identity(nc, ident[:])
            w1_sb = constp.tile([XD, F1], BF16)
            nc.sync.dma_start(w1_sb[:], W1_in[:, :])
            w1e_sb = constp.tile([F1 + 1, F1], BF16)
            nc.sync.dma_start(w1e_sb[:], W1e_in[:, :])
            b1_sb = constp.tile([F1, 1], BF16)
            nc.sync.dma_start(b1_sb[:], b1_in[:, :])
            w2_sb = constp.tile([F1 + 1, F2], BF16)
            nc.sync.dma_start(w2_sb[:], W2e_in[:, :])
            w3a_sb = constp.tile([128, F3], BF16)
            nc.sync.dma_start(w3a_sb[:], W3a_in[:, :])
            w3b_sb = constp.tile([F2 - 128 + 1, F3], BF16)
            nc.sync.dma_start(w3b_sb[:], W3b_in[:, :])
            zrow = constp.tile([1, EP], BF16)
            nc.vector.memset(zrow[:], 0.0)
            nc.sync.dma_start(h3t[0:1, :], zrow[:])

            # ---- node-major transform: psum tile -> staged table rows ----
            def nm_transform(dest_v, elem, srcs, relu, scale, t_lo, t_hi,
                             fm=None, row_grp=None):
                """For node tiles [t_lo, t_hi): accumulate
                psum[128, F_out] = sum_i srcs[i].lhsT_chunk @ srcs[i].rhs,
                then one Activation (relu?, x dinv?) into the staged-write
                buffer; DMA each TGRP group into the tile-major dest view.
                fm(c0, c1) optionally emits feature-major work for the same
                column range (pipelined).  srcs: list of (slab, k, rhs, f0,
                f1) -> psum[:, f0:f1] accumulates slab[:k, cols]^T @ rhs."""
                with (
                    tc.tile_pool(name="nms", bufs=4) as nsp,
                    tc.tile_pool(name="nmp", bufs=4, space="PSUM") as npp,
                ):
                    for t0 in range(t_lo, t_hi, TG):
                        t1 = min(t0 + TG, t_hi)
                        if fm is not None:
                            fm(t0 * 128, t1 * 128)
                        stg = nsp.tile([128, TG, elem], BF16, tag="stg")
                        if (t0 - t_lo) // TG < 4:
                            nc.vector.memset(stg[:], 0.0)
                        for t in range(t0, t1):
                            cs = slice(t * 128, (t + 1) * 128)
                            fout = srcs[-1][4]
                            ps = npp.tile([128, fout], F32, tag="ps")
                            for i, (slab, kk, rhs, f0, f1) in enumerate(srcs):
                                nc.tensor.matmul(
                                    ps[:, f0:f1], slab[:kk, cs], rhs,
                                    start=(i == 0), stop=(i == len(srcs) - 1))
                            sc = dcol_sb[:, t:t + 1] if scale else 1.0
                            nc.scalar.activation(
                                stg[:, t - t0, 0:fout], ps[:],
                                REL if relu else CPY, scale=sc)
                        nc.sync.dma_start(dest_v[:, t0:t1, :],
                                          stg[:, : t1 - t0, :])

            # ---- stage 0: B0 rows = (dinv*x) @ W1, plus f-major zT --------
            zT = chainp.tile([128, N_PAD], BF16, name="zT", tag="chain")
            with (
                tc.tile_pool(name="x1p", bufs=1) as x1p,
                tc.tile_pool(name="z0p", bufs=4, space="PSUM") as zpp,
            ):
                x1_sb = x1p.tile([XD, N_PAD], BF16)
                nc.sync.dma_start(x1_sb[:], X1T_in[:, :])

                def fm0(c0, c1):
                    for cc0 in range(c0, c1, CHUNK):
                        cc1 = min(cc0 + CHUNK, c1)
                        zp = zpp.tile([F1, CHUNK], F32, tag="zp")
                        nc.tensor.matmul(zp[:, : cc1 - cc0], w1_sb[:],
                                         x1_sb[:, cc0:cc1], start=True,
                                         stop=True)
                        nc.scalar.activation(zT[:F1, cc0:cc1],
                                             zp[:, : cc1 - cc0], CPY)

                nm_transform(tview(B0, E1), E1,
                             [(x1_sb, XD, w1_sb[:], 0, F1)],
                             relu=False, scale=False, t_lo=0, t_hi=NTIL,
                             fm=fm0)
            c1 = min(512, E_cap // 16)
            nc.sync.dma_start(idx_sb[:, :c1], idx_in[:, :c1])
            nc.sync.dma_start(rel_sb[:], rel_in[:, :])
            nc.sync.dma_start(sval_sb[:], sval_in[:, :])
            nc.sync.dma_start(idx_sb[:, c1:], idx_in[:, c1:])
            nc.sync.dma_start(slot_sb[:], slot_in[:, :])

            def reduce_scatter(P, A):
                nc.gpsimd.collective_compute(
                    "ReduceScatter", mybir.AluOpType.add,
                    replica_groups=[list(range(NC))],
                    ins=[P[:, :, :].opt()], outs=[A[:, :].opt()])

            # ---- aggregation layer -------------------------------------
            def conv_agg(li, B_in, ELEM_in, flo, fhi, P_hs, A_hs):
                """Gather from B_in, scatter-matmul into variable-width
                global windows (half-major stream order), write feature-major
                partials into P_hs[h] [NC, flo+fhi, HCOL]; the half-0
                ReduceScatter is issued RS_DELAY gather-calls after its last
                window so Pool-side desc-gen never stalls behind it."""
                WSTART, WWID = meta["WSTART"], meta["WWID"]
                WCB, WH = meta["WCB"], meta["WH"]
                W_HALF = int(np.searchsorted(WH, 1))
                STGC = 2048
                ngath = _ceil(NBLK, GBLK)
                with (
                    tc.tile_pool(name=f"gb{li}", bufs=7) as gbp,
                    tc.tile_pool(name=f"st{li}", bufs=10) as stp,
                    tc.tile_pool(name=f"wg{li}", bufs=4) as wgp,
                    tc.tile_pool(name=f"ap{li}", bufs=(4 if not fhi else 3),
                                 space="PSUM") as aps,
                    tc.tile_pool(name=f"ah{li}", bufs=3, space="PSUM") as ahs,
                ):
                    gtiles = {}
                    cur = dict(key=None, used=0, col0=0)
                    w = 0
                    pblk = 0
                    issued = 0
                    rs0_due = None

                    def flush():
                        if cur["key"] is None or cur["used"] == 0:
                            return
                        fcb, fh = cur["key"]
                        cs = slice(cur["col0"], cur["col0"] + cur["used"])
                        nc.sync.dma_start(P_hs[fh][fcb, :flo, cs],
                                          cur["lo"][:, : cur["used"]])
                        if fhi:
                            nc.sync.dma_start(P_hs[fh][fcb, flo:flo + fhi, cs],
                                              cur["hi"][:, : cur["used"]])
                        cur["key"] = None
                        cur["used"] = 0

                    def do_window(w, pblk):
                        kw = int(K[w])
                        wid = int(WWID[w])
                        cb, h = int(WCB[w]), int(WH[w])
                        pcol = int(WSTART[w]) - cb * N_PAD - h * HCOL
                        ps = aps.tile([flo, WIN], F32, tag="ps")
                        ps_hi = None
                        if fhi:
                            ps_hi = ahs.tile([fhi, WIN], F32, tag="psh")
                        for j in range(kw):
                            b = pblk + j
                            gt = gtiles[b // GBLK]
                            ch = b % GBLK
                            st = stp.tile([128, WIN], BF16, tag="st")
                            nc.vector.tensor_scalar(
                                st[:, :wid], iota_sb[:, :wid],
                                rel_sb[:, b:b + 1],
                                sval_sb[:, b:b + 1], EQ, MUL)
                            nc.tensor.matmul(
                                ps[:, :wid], gt[:, ch, :flo], st[:, :wid],
                                start=(j == 0), stop=(j == kw - 1))
                            if fhi:
                                nc.tensor.matmul(
                                    ps_hi[:, :wid], gt[:, ch, flo:flo + fhi],
                                    st[:, :wid],
                                    start=(j == 0), stop=(j == kw - 1))
                        # stage into the column-accumulating write group
                        if (cur["key"] != (cb, h)
                                or cur["used"] + wid > STGC):
                            flush()
                        if cur["key"] is None:
                            cur["key"] = (cb, h)
                            cur["col0"] = pcol
                            cur["lo"] = wgp.tile([flo, STGC], FP8,
                                                 name="stg_lo", tag="sl")
                            if fhi:
                                cur["hi"] = wgp.tile([fhi, STGC], FP8,
                                                     name="stg_hi", tag="sh")
                        u = cur["used"]
                        nc.scalar.activation(cur["lo"][:, u:u + wid],
                                             ps[:, :wid], CPY)
                        if fhi:
                            nc.vector.tensor_scalar(
                                cur["hi"][:, u:u + wid], ps_hi[:, :wid],
                                1.0, None, MUL)
                        cur["used"] = u + wid

                    for g in range(ngath + 1):
                        if g < ngath:
                            nb = min(GBLK, NBLK - g * GBLK)
                            gt = gbp.tile([128, GBLK, ELEM_in], BF16, tag="gb")
                            c0 = g * GBLK * 8
                            nc.gpsimd.dma_gather(
                                gt[:, :nb, :], B_in[:, :],
                                idx_sb[:, c0:c0 + nb * 8],
                                nb * 128, nb * 128, ELEM_in)
                            gtiles[g] = gt
                            issued += nb
                        if rs0_due is not None and g >= rs0_due:
                            reduce_scatter(P_hs[0], A_hs[0])
                            rs0_due = None
                        while (w < NWG
                               and pblk + int(K[w]) <= issued):
                            do_window(w, pblk)
                            pblk += int(K[w])
                            w += 1
                            if w == W_HALF:
                                flush()
                                rs0_due = g + RS_DELAY
                    if rs0_due is not None:
                        reduce_scatter(P_hs[0], A_hs[0])
                    flush()
                    reduce_scatter(P_hs[1], A_hs[1])

            # ---- L1 ----
            if SL >= 2:
                conv_agg(1, B0, E1, F1, 0, P1h, A1h)
            # h1 = relu(A1 + dinv*z + b1); B1 rows = dinv*h1 (via [I;b1])
            if SL >= 4:
              h1T = chainp.tile([128, N_PAD], BF16, name="h1T", tag="chain")
              with tc.tile_pool(name="t1p", bufs=1) as t1p:
                  d1_sb = dbcp.tile([128, N_PAD], BF16, name="d1", tag="dbc")
                  nc.sync.dma_start(d1_sb[:], d1bc_in[:, :])
                  slab = t1p.tile([F1 + 1, N_PAD], BF16, tag="slab1")
                  nc.vector.memset(slab[:], 1.0)
                  nc.vector.tensor_tensor(zT[:F1, :], zT[:F1, :], d1_sb[:F1, :],
                                          MUL)

                  def fm1(c0, c1):
                      nc.scalar.activation(h1T[:F1, c0:c1], slab[:F1, c0:c1],
                                           REL, bias=b1_sb[:, 0:1])
                  for h in range(2):
                      cs = slice(h * HCOL, (h + 1) * HCOL)
                      a8 = t1p.tile([F1, HCOL], FP8, name="a8_1", tag="a81")
                      nc.sync.dma_start(a8[:], A1h[h][:, :])
                      nc.vector.tensor_scalar(slab[:F1, cs], a8[:], 1.0, None,
                                              MUL)
                      nc.vector.tensor_tensor(slab[:F1, cs], slab[:F1, cs],
                                              zT[:F1, cs], ADD)
                      nm_transform(tview(B1, E1), E1,
                                   [(slab, F1 + 1, w1e_sb[:], 0, F1)],
                                   relu=True, scale=True,
                                   t_lo=h * HTIL, t_hi=(h + 1) * HTIL, fm=fm1)

            # ---- L2 ----
            if SL >= 5:
                conv_agg(2, B1, E1, F1, 0, P2h, A2h)
            # h2 = relu((A2 + dinv^2*h1 ; 1) @ W2e); B2 rows = dinv*h2
            if SL >= 6:
              d2_sb = dbcp.tile([128, N_PAD], BF16, name="d2", tag="dbc")
              nc.sync.dma_start(d2_sb[:], d2bc_in[:, :])
              h2a = chainp.tile([128, N_PAD], BF16, name="h2a", tag="chain")
              h2b = chainp.tile([128, N_PAD], BF16, name="h2b", tag="chain")
              with (
                  tc.tile_pool(name="t2p", bufs=1) as t2p,
                  tc.tile_pool(name="t2ps", bufs=2, space="PSUM") as t2pp,
              ):
                  slab = t2p.tile([F1 + 1, N_PAD], BF16, tag="slab2")
                  nc.vector.memset(slab[:], 1.0)
                  nc.vector.tensor_tensor(h1T[:F1, :], h1T[:F1, :],
                                          d2_sb[:F1, :], MUL)

                  def fm2(g0, g1):
                      for c0 in range(g0, g1, CHUNK):
                          c1 = min(c0 + CHUNK, g1)
                          cw = c1 - c0
                          tpa = t2pp.tile([128, CHUNK], F32, tag="tpa",
                                          name="tpa")
                          nc.tensor.matmul(tpa[:, :cw], w2_sb[:, 0:128],
                                           slab[:, c0:c1], start=True,
                                           stop=True)
                          nc.scalar.activation(h2a[:, c0:c1], tpa[:, :cw],
                                               REL)
                          tpb = t2pp.tile([F2 - 128, CHUNK], F32, tag="tpb",
                                          name="tpb")
                          nc.tensor.matmul(tpb[:, :cw], w2_sb[:, 128:F2],
                                           slab[:, c0:c1], start=True,
                                           stop=True)
                          nc.scalar.activation(h2b[: F2 - 128, c0:c1],
                                               tpb[:, :cw], REL)
                  for h in range(2):
                      cs = slice(h * HCOL, (h + 1) * HCOL)
                      a8 = t2p.tile([F1, HCOL], FP8, name="a8_2", tag="a82")
                      nc.sync.dma_start(a8[:], A2h[h][:, :])
                      nc.vector.tensor_scalar(slab[:F1, cs], a8[:], 1.0, None,
                                              MUL)
                      nc.vector.tensor_tensor(slab[:F1, cs], slab[:F1, cs],
                                              h1T[:F1, cs], ADD)
                      nm_transform(tview(B2, E2), E2,
                                   [(slab, F1 + 1, w2_sb[:], 0, F2)],
                                   relu=True, scale=True,
                                   t_lo=h * HTIL, t_hi=(h + 1) * HTIL, fm=fm2)

            # ---- L3 ----
            if SL >= 7:
                conv_agg(3, B2, E2, 128, F2 - 128, P3h, A3h)
            # h3 = relu((A3 + dinv^2*h2 ; 1) @ W3); h3t rows = h3 (no dinv)
            if SL >= 8:
              nhi = F2 - 128
              with (
                  tc.tile_pool(name="t3p", bufs=1) as t3p,
              ):
                  slab_lo = t3p.tile([128, N_PAD], BF16, tag="slab3a")
                  slab_hi = t3p.tile([nhi + 1, N_PAD], BF16, tag="slab3b")
                  nc.vector.memset(slab_hi[:], 1.0)
                  nc.vector.tensor_tensor(h2a[:], h2a[:], d2_sb[:, :], MUL)
                  nc.vector.tensor_tensor(h2b[:nhi, :], h2b[:nhi, :],
                                          d2_sb[:nhi, :], MUL)
                  for h in range(2):
                      cs = slice(h * HCOL, (h + 1) * HCOL)
                      a8l = t3p.tile([128, HCOL], FP8, name="a8_3l",
                                     tag="a83l")
                      nc.sync.dma_start(a8l[:], A3h[h][0:128, :])
                      nc.vector.tensor_scalar(slab_lo[:, cs], a8l[:], 1.0,
                                              None, MUL)
                      nc.vector.tensor_tensor(slab_lo[:, cs], slab_lo[:, cs],
                                              h2a[:, cs], ADD)
                      a8h = t3p.tile([nhi, HCOL], FP8, name="a8_3h",
                                     tag="a83h")
                      nc.sync.dma_start(a8h[:], A3h[h][128:F2, :])
                      nc.vector.tensor_scalar(slab_hi[:nhi, cs], a8h[:], 1.0,
                                              None, MUL)
                      nc.vector.tensor_tensor(slab_hi[:nhi, cs],
                                              slab_hi[:nhi, cs],
                                              h2b[:nhi, cs], ADD)
                      nm_transform(h3m, EP,
                                   [(slab_lo, 128, w3a_sb[:], 0, F3),
                                    (slab_hi, nhi + 1, w3b_sb[:], 0, F3)],
                                   relu=True, scale=False,
                                   t_lo=h * HTIL, t_hi=(h + 1) * HTIL)

            # ---- pooling: slot gather + 48-wide reduce_max ----
            FCH = [(0, 128), (128, 256), (256, F3)]
            if SL >= 9:
              gT = holdp.tile([128, 3 * GPC], BF16, name="gT")
              PG = 6             # slot-tiles per gather: 768 idxs = 16 graphs
              assert (128 * PG) % SLOT == 0 and (NSLOT // 128) % PG == 0
              GPR = 128 * PG // SLOT  # graphs per gather/reduce group
              with (
                  tc.tile_pool(name="poolg", bufs=3) as pgp,
                  tc.tile_pool(name="poolps", bufs=2, space="PSUM") as pps,
              ):
                  NT = NSLOT // 128
                  for tg in range(0, NT, PG):
                      pt = pgp.tile([128, PG, EP], BF16, tag="pg")
                      nc.gpsimd.dma_gather(
                          pt[:, :, :], h3t[:, :],
                          slot_sb[:, tg * 8:(tg + PG) * 8],
                          PG * 128, PG * 128, EP)
                      gcol = tg * 128 // SLOT
                      for i, (f0, f1) in enumerate(FCH):
                          csz = f1 - f0
                          tps = pps.tile([128, PG * 128], BF16,
                                         tag=f"tps{i}", name=f"tps{i}")
                          for t in range(PG):
                              nc.tensor.transpose(
                                  tps[:csz, t * 128:(t + 1) * 128],
                                  pt[:, t, i * 128:i * 128 + csz],
                                  ident[:])
                          nc.vector.tensor_reduce(
                              gT[:csz, i * GPC + gcol:
                                 i * GPC + gcol + GPR],
                              tps[:csz, :].rearrange(
                                  "p (g s) -> p g s", s=SLOT),
                              mybir.AxisListType.X, MAX)

            # ---- MLP (feature-major, bf16) ----
            if SL >= 10:
              with (
                  tc.tile_pool(name="mlpw", bufs=1) as mwp,
                  tc.tile_pool(name="mlps", bufs=1) as msp,
                  tc.tile_pool(name="mlpps", bufs=6, space="PSUM") as mps,
              ):
                  ksz = [128, 128, F3 - 256]
                  wg1_sb = [mwp.tile([ksz[i], D1], BF16, name=f"wg1_{i}",
                                     tag=f"wg1_{i}") for i in range(3)]
                  for i in range(3):
                      r0 = 128 * i
                      nc.sync.dma_start(wg1_sb[i][:], Wg1_in[r0:r0 + ksz[i], :])
                  bg1_sb = msp.tile([128, D1 // 128], BF16)
                  nc.sync.dma_start(bg1_sb[:], bg1_in[:, :])
                  wg2_sb = [mwp.tile([128, D2], BF16, name=f"wg2_{i}",
                                     tag=f"wg2_{i}") for i in range(D1 // 128)]
                  for i in range(D1 // 128):
                      nc.sync.dma_start(wg2_sb[i][:],
                                        Wg2_in[i * 128:(i + 1) * 128, :])
                  bg2_sb = msp.tile([128, 1], F32)
                  nc.sync.dma_start(bg2_sb[:], bg2_in[:, :])
                  wf1_sb = mwp.tile([128, D3], BF16)
                  nc.sync.dma_start(wf1_sb[:], Wf1_in[:, :])
                  bf1_sb = msp.tile([128, D3 // 128], BF16)
                  nc.sync.dma_start(bf1_sb[:], bf1_in[:, :])
                  wf2_sb = [mwp.tile([128, D4], BF16, name=f"wf2_{i}",
                                     tag=f"wf2_{i}") for i in range(D3 // 128)]
                  for i in range(D3 // 128):
                      nc.sync.dma_start(wf2_sb[i][:],
                                        Wf2_in[i * 128:(i + 1) * 128, :])
                  bf2_sb = msp.tile([128, D4 // 128], BF16)
                  nc.sync.dma_start(bf2_sb[:], bf2_in[:, :])
                  wo_sb = [mwp.tile([128, 1], BF16, name=f"wo_{i}",
                                    tag=f"wo_{i}") for i in range(D4 // 128)]
                  for i in range(D4 // 128):
                      nc.sync.dma_start(wo_sb[i][:],
                                        Wo_in[i * 128:(i + 1) * 128, :])
                  bo_sb = msp.tile([1, 1], BF16)
                  nc.sync.dma_start(bo_sb[:], bo_in[:, :])

                  def mlp_half(g0, g1w, sfx):
                      gw = g1w - g0
                      g1t = msp.tile([128, (D1 // 128) * gw], BF16,
                                     name=f"g1{sfx}", tag=f"g1{sfx}")
                      for m in range(D1 // 128):
                          ps = mps.tile([128, gw], F32, tag="mlp", name="ps")
                          for i in range(3):
                              nc.tensor.matmul(
                                  ps[:], wg1_sb[i][:, m * 128:(m + 1) * 128],
                                  gT[:ksz[i], i * GPC + g0: i * GPC + g1w],
                                  start=(i == 0), stop=(i == 2))
                          nc.scalar.activation(g1t[:, m * gw:(m + 1) * gw],
                                               ps[:], REL,
                                               bias=bg1_sb[:, m:m + 1])
                      ps = mps.tile([128, gw], F32, tag="mlp", name="ps")
                      for i in range(D1 // 128):
                          nc.tensor.matmul(ps[:], wg2_sb[i][:],
                                           g1t[:, i * gw:(i + 1) * gw],
                                           start=(i == 0),
                                           stop=(i == D1 // 128 - 1))
                      g2 = msp.tile([128, gw], BF16, name=f"g2{sfx}",
                                    tag=f"g2{sfx}")
                      nc.vector.tensor_scalar(g2[:], ps[:], bg2_sb[:, 0:1],
                                              None, ADD)
                      c1t = msp.tile([128, (D3 // 128) * gw], BF16,
                                     name=f"c1{sfx}", tag=f"c1{sfx}")
                      for m in range(D3 // 128):
                          ps = mps.tile([128, gw], F32, tag="mlp", name="ps")
                          nc.tensor.matmul(
                              ps[:], wf1_sb[:, m * 128:(m + 1) * 128],
                              g2[:], start=True, stop=True)
                          nc.scalar.activation(c1t[:, m * gw:(m + 1) * gw],
                                               ps[:], REL,
                                               bias=bf1_sb[:, m:m + 1])
                      c2t = msp.tile([128, (D4 // 128) * gw], BF16,
                                     name=f"c2{sfx}", tag=f"c2{sfx}")
                      for m in range(D4 // 128):
                          ps = mps.tile([128, gw], F32, tag="mlp", name="ps")
                          for i in range(D3 // 128):
                              nc.tensor.matmul(
                                  ps[:], wf2_sb[i][:, m * 128:(m + 1) * 128],
                                  c1t[:, i * gw:(i + 1) * gw],
                                  start=(i == 0), stop=(i == D3 // 128 - 1))
                          nc.scalar.activation(c2t[:, m * gw:(m + 1) * gw],
                                               ps[:], REL,
                                               bias=bf2_sb[:, m:m + 1])
                      pso = mps.tile([1, gw], F32, tag="mlpo", name="pso",
                                     bufs=1)
                      for i in range(D4 // 128):
                          nc.tensor.matmul(pso[:], wo_sb[i][:],
                                           c2t[:, i * gw:(i + 1) * gw],
                                           start=(i == 0),
                                           stop=(i == D4 // 128 - 1))
                      o_sb = msp.tile([1, gw], F32, name=f"o{sfx}",
                                      tag=f"o{sfx}")
                      nc.scalar.activation(o_sb[:], pso[:], SIG,
                                           bias=bo_sb[:, 0:1])
                      nc.sync.dma_start(out_d[:, g0:g1w], o_sb[:])

                  mlp_half(0, GPC, "a")
            if SL < 10:
                with tc.tile_pool(name="zo", bufs=1) as zop:
                    zo = zop.tile([1, GPC], F32)
                    nc.vector.memset(zo[:], 0.0)
                    nc.sync.dma_start(out_d[:, :], zo[:])


    nc.compile()
    return nc


LAST_EXEC_NS = None
LAST_RES = None


def kernel(**inputs):
    global LAST_EXEC_NS, LAST_RES
    x = np.asarray(inputs["x"])
    edge_index = np.asarray(inputs["edge_index"])
    batch = np.asarray(inputs["batch"])
    weights = {k: np.asarray(v) for k, v in inputs.items()
               if k not in ("x", "edge_index", "batch")}
    meta, in_maps = _plan(x, edge_index, batch, weights)
    nc = _build(meta)
    res = run_bass_kernel_spmd(nc, in_maps, core_ids=list(range(NC)))
    LAST_RES = res
    LAST_EXEC_NS = res.exec_time_ns
    out = np.concatenate(
        [np.asarray(res.results[c]["out_d"][0], np.float32)
         for c in range(NC)])
    return out.reshape(-1, 1)


# revision 56
# speedup vs baseline: 1.0391x; 1.0391x over previous
"""GCN (3x GCNConv + global max pool + MLP) on 8 Trainium2 NeuronCores.

Strategy (v3 — node-major transforms + tile-major tables + delayed RS):
  - Nodes blocked by graph ownership: core c owns contiguous local rows
    [0, N_PAD); global padded row = c*N_PAD + r.
  - Edges are assigned to the core owning their SRC node, so every gather
    reads the core-LOCAL bounce table (int16 indices, single table).
  - Each layer: gather src rows (bf16) -> one-hot S matmuls (bf16)
    accumulate [F, WIN]-window partials over the GLOBAL dst space ->
    partial table P[c_blk, F, N_PAD] fp8 -> ReduceScatter(add) gives each
    core its fully-reduced agg [F, N_PAD].  RS for half 0 is ISSUED a few
    gather-calls after its windows complete so the Pool sequencer (which
    owns both SWDGE desc-gen and the collective dispatch) never stalls
    desc-gen behind the collective's input semaphores.
  - Self-loops are the post-RS diagonal term (dinv^2 * h_prev), added
    feature-major into the transform slab.
  - Transform + next-layer bounce table in ONE pass: for each 128-node
    tile, matmul(lhsT=slab[:,tile], rhs=[W;b]) yields a node-major psum
    tile [128, F_out]; one Activation op applies relu, scales by the
    per-node dinv (scale-ptr), converts to bf16 and lands directly in the
    staged write buffer.  No transpose pass, no separate rescale.
  - Bounce tables are TILE-MAJOR (row of node v = (v%128)*NTIL + v//128)
    so staged writes are >=1.2KB-contiguous per partition (no 2x small-
    descriptor DMA penalty); gather indices are host-remapped to match.
  - L1 pre-transforms x@W1 on-device (z = (dinv*x)@W1 commutes with agg);
    its post-agg "transform" is the identity+bias matmul [I;b1].
  - Pooling: slot-gather (48 slots/graph) from tile-major h3t with 768B
    rows + transpose + batched reduce_max.  MLP feature-major bf16.
"""
import os
import sys

sys.path.insert(0, "/opt/trn_rl_repo")

import numpy as np

import concourse.bass as bass
import concourse.mybir as mybir
import concourse.tile as tile
from concourse import bacc
from concourse.bass_utils import run_bass_kernel_spmd
from concourse.masks import make_identity
from concourse.tile_rust import add_dep_helper

F32 = mybir.dt.float32
BF16 = mybir.dt.bfloat16
FP8 = mybir.dt.float8e4
I16 = mybir.dt.int16
NC = 8
WIN = 256          # max dst-window width (S-matmul free size)
SLOT = 48          # pooling slots per graph (max graph = 46)
GBLK = 8           # blocks per dma_gather call (1024-idx HW ring limit)
CHUNK = 512        # feature-major column chunk (psum-bank bound)
TGRP = 5           # node-tiles per staged table-write group
RS_DELAY = 6       # gather calls between half-0 window completion and RS issue
INJ_LAG = 16       # gather calls between RS-h0 issue and transform injection

try:
    import ml_dtypes
    _BF = ml_dtypes.bfloat16
except ImportError:  # pragma: no cover
    _BF = np.float32


def _ceil(a, b):
    return -(-a // b)


def _bf(x):
    return np.asarray(x, np.float32).astype(_BF)


def _wrap_idx(flat):
    # dma_gather index layout: idx i -> partition i%16, col i//16, replicated x8
    w = flat.reshape(-1, 16).T.astype(np.int16)
    return np.tile(w, (8, 1))


def _plan(x, edge_index, batch, weights):
    N, XD = x.shape
    G = 2048 if N == 50000 else int(batch.max()) + 1
    assert G % NC == 0
    GPC = G // NC

    batch = np.asarray(batch, dtype=np.int64)
    sizes = np.bincount(batch, minlength=G)
    assert sizes.min() >= 1 and sizes.max() <= SLOT
    gcore = np.arange(G) // GPC
    node_core = gcore[batch]

    core_start = np.searchsorted(batch, np.arange(NC) * GPC)
    core_start = np.concatenate([core_start, [N]])
    ncounts = np.diff(core_start)
    N_PAD = _ceil(int(ncounts.max()), WIN) * WIN
    assert N_PAD % 256 == 0 and N_PAD < 32768
    NTIL = N_PAD // 128

    local_row = np.arange(N) - core_start[node_core]
    g_row = node_core * N_PAD + local_row

    # tile-major remap: node local row r -> table row (r%128)*NTIL + r//128
    def remap(r):
        return (r % 128) * NTIL + r // 128

    src = np.asarray(edge_index[0], dtype=np.int64)
    dst = np.asarray(edge_index[1], dtype=np.int64)
    deg = (np.bincount(dst, minlength=N) + 1).astype(np.float32)
    dinv = (1.0 / np.sqrt(deg)).astype(np.float32)

    # real edges only; self-loops become the post-RS diagonal add
    e_core = node_core[src]
    e_idx = remap(local_row[src]).astype(np.int16)
    e_sval = dinv[dst]

    # variable-width windows: greedy boundaries per (half, core-block)
    # region so each window's max-core edge count just fills K_T blocks
    # (stream order = all half-0 regions first, so RS half 0 fires mid-layer)
    HCOL = N_PAD // 2
    gdst = g_row[dst]
    cnt_cr = np.zeros((NC, NC * N_PAD), np.int32)
    for c in range(NC):
        cnt_cr[c] = np.bincount(gdst[e_core == c], minlength=NC * N_PAD)
    CAP_E, CAP_W = 128 * 3, WIN
    wstart, wwid, wcb, wh, Klist = [], [], [], [], []
    warr = np.zeros(NC * N_PAD, np.int64)
    for h in range(2):
        for cb in range(NC):
            r0 = cb * N_PAD + h * HCOL
            run = np.zeros(NC, np.int64)
            w0 = r0
            for r in range(r0, r0 + HCOL):
                nxt = run + cnt_cr[:, r]
                if r > w0 and (nxt.max() > CAP_E or r - w0 >= CAP_W):
                    wstart.append(w0); wwid.append(r - w0)
                    wcb.append(cb); wh.append(h)
                    Klist.append(max(1, _ceil(int(run.max()), 128)))
                    warr[w0:r] = len(wstart) - 1
                    w0 = r
                    run = cnt_cr[:, r].astype(np.int64)
                else:
                    run = nxt
            wstart.append(w0); wwid.append(r0 + HCOL - w0)
            wcb.append(cb); wh.append(h)
            Klist.append(max(1, _ceil(int(run.max()), 128)))
            warr[w0:r0 + HCOL] = len(wstart) - 1
    NWG = len(wstart)
    K = np.asarray(Klist, np.int64)
    wstart = np.asarray(wstart, np.int64)
    wwid = np.asarray(wwid, np.int64)
    assert wwid.max() <= WIN
    NBLK = int(K.sum())
    E_cap = NBLK * 128
    e_w = warr[gdst]
    e_rel = (gdst - wstart[e_w]).astype(np.float32)
    assert e_rel.min() >= 0 and e_rel.max() < WIN
    key = e_core * NWG + e_w
    blk_off = np.concatenate([[0], np.cumsum(K)[:-1]])

    order = np.lexsort((e_w, e_core))
    s_key = key[order]
    s_idx = e_idx[order]
    s_rel = e_rel[order]
    s_sval = e_sval[order]
    grp_starts = np.searchsorted(s_key, np.arange(NC * NWG))
    grp_ends = np.concatenate([grp_starts[1:], [len(s_key)]])

    idx_w, rel_cols, sval_cols = [], [], []
    for c in range(NC):
        idx16_s = np.zeros(E_cap, np.int16)
        rel_s = np.full(E_cap, -1.0, np.float32)
        sval_s = np.zeros(E_cap, np.float32)
        a = grp_starts[c * NWG:(c + 1) * NWG]
        b = grp_ends[c * NWG:(c + 1) * NWG]
        for w in range(NWG):
            m = b[w] - a[w]
            if m == 0:
                continue
            d0 = blk_off[w] * 128
            idx16_s[d0:d0 + m] = s_idx[a[w]:b[w]]
            rel_s[d0:d0 + m] = s_rel[a[w]:b[w]]
            sval_s[d0:d0 + m] = s_sval[a[w]:b[w]]
        idx_w.append(_wrap_idx(idx16_s))
        rel_cols.append(np.ascontiguousarray(rel_s.reshape(NBLK, 128).T))
        sval_cols.append(np.ascontiguousarray(sval_s.reshape(NBLK, 128).T))

    # pooling slot plan: graphs are SORTED by size (descending) per core and
    # packed into bins of GBIN graphs; bin b pads every graph to the max size
    # any core has at those sorted positions (order statistics align across
    # cores, so shared padding wastes little).  gT column = sorted position;
    # the host unpermutes the final output.
    GBIN = 16
    NBIN = GPC // GBIN
    gstart = np.concatenate([[0], np.cumsum(sizes)])
    order_c = []          # per core: ids size-sorted within each index half
    for c in range(NC):
        sz = sizes[c * GPC:(c + 1) * GPC]
        h = GPC // 2
        order_c.append(np.concatenate([
            np.argsort(-sz[:h], kind="stable"),
            h + np.argsort(-sz[h:], kind="stable")]))
    pad_b = np.zeros(NBIN, np.int64)
    for b in range(NBIN):
        for c in range(NC):
            ids = order_c[c][b * GBIN:(b + 1) * GBIN]
            pad_b[b] = max(pad_b[b], int(sizes[c * GPC + ids].max()))
    nidx_b = [int(_ceil(GBIN * pad_b[b], 128) * 128) for b in range(NBIN)]
    off_b = np.concatenate([[0], np.cumsum(nidx_b)]).astype(np.int64)
    NSLOT = int(off_b[-1])
    slot_w = []
    pgmaxt = []
    for c in range(NC):
        sl = np.full(NSLOT, -1, np.int64)
        for b in range(NBIN):
            for k in range(GBIN):
                gj = int(order_c[c][b * GBIN + k])
                gi = c * GPC + gj
                st0 = gstart[gi] - core_start[c]
                sz = int(sizes[gi])
                o = int(off_b[b]) + k * int(pad_b[b])
                sl[o:o + sz] = np.arange(st0, st0 + sz) + 1
        sl[sl < 0] = 0
        slot_w.append(_wrap_idx(sl.astype(np.int16)))
    for b in range(NBIN):
        mt = 1
        for c in range(NC):
            s = slot_w  # placeholder
        # max row any core needs in bin b (+1)
        mt = 1
        for c in range(NC):
            ids = order_c[c][b * GBIN:(b + 1) * GBIN]
            for gj in ids:
                gi = c * GPC + int(gj)
                mt = max(mt, int(gstart[gi + 1] - core_start[c]) + 1)
        pgmaxt.append(mt)
    unperm = np.concatenate(
        [c * GPC + order_c[c] for c in range(NC)])  # gT order -> graph id

    # per-core node data
    xs = (dinv[:, None] * np.asarray(x, np.float32)).astype(np.float32)
    X1T, d1bc, d2bc, dcol = [], [], [], []
    for c in range(NC):
        n0, n1 = core_start[c], core_start[c + 1]
        xt = np.zeros((XD, N_PAD), np.float32)
        xt[:, : n1 - n0] = xs[n0:n1].T
        X1T.append(_bf(xt))
        dl = np.zeros(N_PAD, np.float32)
        dl[: n1 - n0] = dinv[n0:n1]
        d1bc.append(_bf(np.broadcast_to(dl[None, :], (128, N_PAD))))
        d2bc.append(_bf(np.broadcast_to((dl * dl)[None, :], (128, N_PAD))))
        dcol.append(np.ascontiguousarray(dl.reshape(NTIL, 128).T))

    W1, b1 = weights["W1"], weights["b1"]
    W2, b2 = weights["W2"], weights["b2"]
    W3, b3 = weights["W3"], weights["b3"]
    F1, F2, F3 = W1.shape[1], W2.shape[1], W3.shape[1]
    assert (XD, F1, F2, F3) == (78, 78, 156, 312)
    w_shared = dict(
        W1=_bf(W1),
        W1e=_bf(np.vstack([np.eye(F1, dtype=np.float32),
                           np.asarray(b1, np.float32)[None, :]])),
        b1c=_bf(np.asarray(b1, np.float32).reshape(-1, 1)),
        W2e=_bf(np.vstack([np.asarray(W2, np.float32),
                           np.asarray(b2, np.float32)[None, :]])),
        W3a=_bf(np.asarray(W3[:128], np.float32)),
        W3b=_bf(np.vstack([np.asarray(W3[128:], np.float32),
                           np.asarray(b3, np.float32)[None, :]])),
        Wg1=_bf(weights["Wg1"]), Wg2=_bf(weights["Wg2"]),
        Wf1=_bf(weights["Wf1"]), Wf2=_bf(weights["Wf2"]),
        Wo=_bf(weights["Wo"]),
        bg1=_bf(np.asarray(weights["bg1"], np.float32).reshape(-1, 128).T),
        bg2=np.asarray(weights["bg2"], np.float32).reshape(-1, 1),
        bf1=_bf(np.asarray(weights["bf1"], np.float32).reshape(-1, 128).T),
        bf2=_bf(np.asarray(weights["bf2"], np.float32).reshape(-1, 128).T),
        bo=_bf(np.asarray(weights["bo"], np.float32).reshape(1, 1)),
        iota=_bf(np.broadcast_to(
            np.arange(WIN, dtype=np.float32)[None, :], (128, WIN))),
    )

    meta = dict(
        N=N, XD=XD, G=G, GPC=GPC, N_PAD=N_PAD, NWG=NWG, NTIL=NTIL,
        K=K, NBLK=NBLK, E_cap=E_cap, NSLOT=NSLOT,
        F1=F1, F2=F2, F3=F3,
        D1=weights["Wg1"].shape[1], D2=weights["Wg2"].shape[1],
        D3=weights["Wf1"].shape[1], D4=weights["Wf2"].shape[1],
        WSTART=wstart, WWID=wwid, WCB=np.asarray(wcb), WH=np.asarray(wh),
        PGMAXT=np.asarray(pgmaxt), PAD_B=pad_b, NIDX_B=nidx_b,
        OFF_B=off_b, NBIN=NBIN, GBIN=GBIN, UNPERM=unperm,
    )
    in_maps = [
        dict(
            X1T=X1T[c], d1bc=d1bc[c], d2bc=d2bc[c], dcol=dcol[c],
            idx_w=idx_w[c], rel_c=rel_cols[c], sval_c=sval_cols[c],
            slot_w=slot_w[c], **w_shared,
        )
        for c in range(NC)
    ]
    return meta, in_maps


def _build(meta):
    N_PAD, NWG, NTIL = meta["N_PAD"], meta["NWG"], meta["NTIL"]
    K, NBLK, E_cap, NSLOT = meta["K"], meta["NBLK"], meta["E_cap"], meta["NSLOT"]
    GPC, XD = meta["GPC"], meta["XD"]
    F1, F2, F3 = meta["F1"], meta["F2"], meta["F3"]
    D1, D2, D3, D4 = meta["D1"], meta["D2"], meta["D3"], meta["D4"]
    E1, E2 = 128, 256            # bounce row elems (bf16): 256B / 512B
    EP = 384                     # h3 row elems (312 -> 384, 768B)
    HCOL = N_PAD // 2
    HTIL = NTIL // 2
    TG = TGRP if HTIL % TGRP == 0 else 1
    assert NSLOT % 128 == 0

    SL = int(os.environ.get("KRS_STOP", "10"))
    nc = bacc.Bacc("TRN2", target_bir_lowering=False, debug=False,
                   num_devices=NC, num_swdge_queues=2)

    # ---- I/O ----
    X1T_in = nc.dram_tensor("X1T", [XD, N_PAD], BF16, kind="ExternalInput")
    d1bc_in = nc.dram_tensor("d1bc", [128, N_PAD], BF16, kind="ExternalInput")
    d2bc_in = nc.dram_tensor("d2bc", [128, N_PAD], BF16, kind="ExternalInput")
    dcol_in = nc.dram_tensor("dcol", [128, NTIL], F32, kind="ExternalInput")
    idx_in = nc.dram_tensor("idx_w", [128, E_cap // 16], I16, kind="ExternalInput")
    rel_in = nc.dram_tensor("rel_c", [128, NBLK], F32, kind="ExternalInput")
    sval_in = nc.dram_tensor("sval_c", [128, NBLK], F32, kind="ExternalInput")
    slot_in = nc.dram_tensor("slot_w", [128, NSLOT // 16], I16, kind="ExternalInput")
    iota_in = nc.dram_tensor("iota", [128, WIN], BF16, kind="ExternalInput")
    W1_in = nc.dram_tensor("W1", [XD, F1], BF16, kind="ExternalInput")
    W1e_in = nc.dram_tensor("W1e", [F1 + 1, F1], BF16, kind="ExternalInput")
    b1_in = nc.dram_tensor("b1c", [F1, 1], BF16, kind="ExternalInput")
    W2e_in = nc.dram_tensor("W2e", [F1 + 1, F2], BF16, kind="ExternalInput")
    W3a_in = nc.dram_tensor("W3a", [128, F3], BF16, kind="ExternalInput")
    W3b_in = nc.dram_tensor("W3b", [F2 - 128 + 1, F3], BF16, kind="ExternalInput")
    Wg1_in = nc.dram_tensor("Wg1", [F3, D1], BF16, kind="ExternalInput")
    Wg2_in = nc.dram_tensor("Wg2", [D1, D2], BF16, kind="ExternalInput")
    Wf1_in = nc.dram_tensor("Wf1", [D2, D3], BF16, kind="ExternalInput")
    Wf2_in = nc.dram_tensor("Wf2", [D3, D4], BF16, kind="ExternalInput")
    Wo_in = nc.dram_tensor("Wo", [D4, 1], BF16, kind="ExternalInput")
    bg1_in = nc.dram_tensor("bg1", [128, D1 // 128], BF16, kind="ExternalInput")
    bg2_in = nc.dram_tensor("bg2", [128, 1], F32, kind="ExternalInput")
    bf1_in = nc.dram_tensor("bf1", [128, D3 // 128], BF16, kind="ExternalInput")
    bf2_in = nc.dram_tensor("bf2", [128, D4 // 128], BF16, kind="ExternalInput")
    bo_in = nc.dram_tensor("bo", [1, 1], BF16, kind="ExternalInput")
    out_d = nc.dram_tensor("out_d", [1, GPC], F32, kind="ExternalOutput")

    REL = mybir.ActivationFunctionType.Relu
    CPY = mybir.ActivationFunctionType.Copy
    SIG = mybir.ActivationFunctionType.Sigmoid
    EQ, MUL, ADD, MAX = (
        mybir.AluOpType.is_equal, mybir.AluOpType.mult,
        mybir.AluOpType.add, mybir.AluOpType.max,
    )

    with tile.TileContext(nc) as tc:
        with (
            tc.tile_pool(name="dramp", bufs=1, space="DRAM") as dramp,
            tc.tile_pool(name="const", bufs=1) as constp,
            tc.tile_pool(name="hold", bufs=1) as holdp,
            tc.tile_pool(name="chain", bufs=3) as chainp,
            tc.tile_pool(name="dbc", bufs=1) as dbcp,
            tc.tile_pool(name="slabs", bufs=2) as slabp,
            tc.tile_pool(name="aux", bufs=1) as auxp,
            tc.tile_pool(name="nsE1", bufs=4) as nsE1,
            tc.tile_pool(name="nsE2", bufs=4) as nsE2,
            tc.tile_pool(name="nsEP", bufs=4) as nsEP,
            tc.tile_pool(name="nmpp", bufs=2, space="PSUM") as nmpp,
        ):
            # DRAM scratch (bounce tables are tile-major: node local row r
            # lives at table row (r%128)*NTIL + r//128)
            B0 = dramp.tile([128 * NTIL, E1], BF16, tag="B0")
            B1 = dramp.tile([128 * NTIL, E1], BF16, tag="B1")
            B2 = dramp.tile([128 * NTIL, E2], BF16, tag="B2")
            P1h = [dramp.tile([NC, F1, HCOL], FP8, name=f"P1{h}",
                              tag=f"P1{h}") for h in range(2)]
            P2h = [dramp.tile([NC, F1, HCOL], FP8, name=f"P2{h}",
                              tag=f"P2{h}") for h in range(2)]
            P3h = [dramp.tile([NC, F2, HCOL], FP8, name=f"P3{h}",
                              tag=f"P3{h}") for h in range(2)]
            A1h = [dramp.tile([F1, HCOL], FP8, name=f"A1{h}",
                              tag=f"A1{h}") for h in range(2)]
            A2h = [dramp.tile([F1, HCOL], FP8, name=f"A2{h}",
                              tag=f"A2{h}") for h in range(2)]
            A3h = [dramp.tile([F2, HCOL], FP8, name=f"A3{h}",
                              tag=f"A3{h}") for h in range(2)]
            h3t = dramp.tile([1 + 128 * NTIL, EP], BF16, tag="h3t")

            def tview(T, elem):
                return T.rearrange("(p t) e -> p t e", t=NTIL)

            def h3m_slice(t0, t1):
                return h3t[1 + t0 * 128: 1 + t1 * 128, :]

            # persistent SBUF (edge streams loaded after stage-0 kickoff)
            idx_sb = holdp.tile([128, E_cap // 16], I16)
            rel_sb = holdp.tile([128, NBLK], F32)
            sval_sb = holdp.tile([128, NBLK], F32)
            slot_sb = holdp.tile([128, NSLOT // 16], I16)
            iota_sb = constp.tile([128, WIN], BF16)
            nc.sync.dma_start(iota_sb[:], iota_in[:, :])
            dcol_sb = constp.tile([128, NTIL], F32)
            nc.sync.dma_start(dcol_sb[:], dcol_in[:, :])
            ident = constp.tile([128, 128], BF16)
            make_identity(nc, ident[:])
            w1_sb = constp.tile([XD, F1], BF16)
            nc.sync.dma_start(w1_sb[:], W1_in[:, :])
            w1e_sb = constp.tile([F1 + 1, F1], BF16)
            nc.sync.dma_start(w1e_sb[:], W1e_in[:, :])
            b1_sb = constp.tile([F1, 1], BF16)
            nc.sync.dma_start(b1_sb[:], b1_in[:, :])
            w2_sb = constp.tile([F1 + 1, F2], BF16)
            nc.sync.dma_start(w2_sb[:], W2e_in[:, :])
            w3a_sb = constp.tile([128, F3], BF16)
            nc.sync.dma_start(w3a_sb[:], W3a_in[:, :])
            w3b_sb = constp.tile([F2 - 128 + 1, F3], BF16)
            nc.sync.dma_start(w3b_sb[:], W3b_in[:, :])
            zrow = constp.tile([1, EP], BF16)
            nc.vector.memset(zrow[:], 0.0)
            nc.sync.dma_start(h3t[0:1, :], zrow[:])

            # ---- node-major transform group: psum tiles -> staged rows ----
            nm_cnt = {}
            AGG = {}

            def gated_dma(dst, srcv, h):
                ld = nc.sync.dma_start(dst, srcv)
                if h == 0 and AGG.get("gate") is not None:
                    add_dep_helper(ld.ins, AGG["gate"].ins, sync=True,
                                   reason="post-RS a8 placement gate")
                return ld

            def nm_group(dest_f, elem, srcs, relu, scale, t0, t1, nsp):
                """For node tiles [t0, t1): accumulate psum[128, F_out] =
                sum_i srcs[i].lhsT_chunk @ srcs[i].rhs, then one Activation
                (relu?, x dinv?) into the staged-write buffer; one DMA into
                the tile-major dest view.  srcs: (slab, k, rhs, f0, f1).
                Staging buffers are zeroed on first rotation only (pad
                columns stay zero forever after)."""
                stg = nsp.tile([128, TG, elem], BF16, tag="stg")
                n = nm_cnt.get(id(nsp), 0)
                if n < 4:
                    nc.vector.memset(stg[:], 0.0)
                nm_cnt[id(nsp)] = n + 1
                for t in range(t0, t1):
                    cs = slice(t * 128, (t + 1) * 128)
                    fout = srcs[-1][4]
                    ps = nmpp.tile([128, F3], F32, tag="ps")
                    for i, (slab, kk, rhs, f0, f1) in enumerate(srcs):
                        nc.tensor.matmul(
                            ps[:, f0:f1], slab[:kk, cs], rhs,
                            start=(i == 0), stop=(i == len(srcs) - 1))
                    sc = dcol_sb[:, t:t + 1] if scale else 1.0
                    nc.scalar.activation(
                        stg[:, t - t0, 0:fout], ps[:, :fout],
                        REL if relu else CPY, scale=sc)
                nc.sync.dma_start(dest_f(t0, t1), stg[:, : t1 - t0, :])

            # ---- stage 0: B0 rows = (dinv*x) @ W1, plus f-major zT --------
            zT = chainp.tile([128, N_PAD], BF16, name="zT", tag="chain")
            with (
                tc.tile_pool(name="x1p", bufs=1) as x1p,
            ):
                x1_sb = x1p.tile([XD, N_PAD], BF16)
                nc.sync.dma_start(x1_sb[:], X1T_in[:, :])
                d1_sb = x1p.tile([128, N_PAD], BF16, name="d1")
                nc.sync.dma_start(d1_sb[:], d1bc_in[:, :])
                for t0 in range(0, NTIL, TG):
                    t1 = min(t0 + TG, NTIL)
                    for cc0 in range(t0 * 128, t1 * 128, CHUNK):
                        cc1 = min(cc0 + CHUNK, t1 * 128)
                        zp = fmpp.tile([F1, CHUNK], F32, tag="tpa")
                        nc.tensor.matmul(zp[:, : cc1 - cc0], w1_sb[:],
                                         x1_sb[:, cc0:cc1], start=True,
                                         stop=True)
                        nc.scalar.activation(zT[:F1, cc0:cc1],
                                             zp[:, : cc1 - cc0], CPY)
                    nm_group(lambda a, b: tview(B0, E1)[:, a:b, :], E1,
                             [(x1_sb, XD, w1_sb[:], 0, F1)],
                             relu=False, scale=False, t0=t0, t1=t1,
                             nsp=nsE1)
                # zT -> d1*zT once (L1 diagonal carries a single dinv)
                nc.vector.tensor_tensor(zT[:F1, :], zT[:F1, :],
                                        d1_sb[:F1, :], MUL)
            c1 = min(512, E_cap // 16)
            nc.sync.dma_start(idx_sb[:, :c1], idx_in[:, :c1])
            nc.sync.dma_start(rel_sb[:], rel_in[:, :])
            nc.sync.dma_start(sval_sb[:], sval_in[:, :])
            nc.sync.dma_start(idx_sb[:, c1:], idx_in[:, c1:])
            nc.sync.dma_start(slot_sb[:], slot_in[:, :])

            def reduce_scatter(P, A):
                nc.gpsimd.collective_compute(
                    "ReduceScatter", mybir.AluOpType.add,
                    replica_groups=[list(range(NC))],
                    ins=[P[:, :, :].opt()], outs=[A[:, :].opt()])

            # ---- aggregation layer -------------------------------------
            def conv_agg(li, B_in, ELEM_in, flo, fhi, P_hs, A_hs,
                         inject=None, inj_lag=INJ_LAG):
                """Gather from B_in, scatter-matmul into variable-width
                global windows (half-major stream order), write feature-major
                partials into P_hs[h] [NC, flo+fhi, HCOL]; the half-0
                ReduceScatter is issued RS_DELAY gather-calls after its last
                window so Pool-side desc-gen never stalls behind it.
                `inject` is a list of thunks (the half-0 post-RS transform)
                drained one per gather call starting INJ_LAG calls after the
                half-0 RS issue, so that work overlaps the half-1 window
                phase instead of queueing behind it."""
                WSTART, WWID = meta["WSTART"], meta["WWID"]
                WCB, WH = meta["WCB"], meta["WH"]
                W_HALF = int(np.searchsorted(WH, 1))
                STGC = 2048
                ngath = _ceil(NBLK, GBLK)
                with (
                    tc.tile_pool(name=f"gb{li}", bufs=7) as gbp,
                    tc.tile_pool(name=f"st{li}", bufs=10) as stp,
                    tc.tile_pool(name=f"wg{li}", bufs=4) as wgp,
                    tc.tile_pool(name=f"ap{li}", bufs=(4 if not fhi else 3),
                                 space="PSUM") as aps,
                    tc.tile_pool(name=f"ah{li}", bufs=3, space="PSUM") as ahs,
                ):
                    gtiles = {}
                    cur = dict(key=None, used=0, col0=0)
                    w = 0
                    pblk = 0
                    issued = 0
                    rs0_due = None
                    inj = list(inject or [])
                    inj_i = 0
                    inj_from = None

                    def flush():
                        if cur["key"] is None or cur["used"] == 0:
                            return
                        fcb, fh = cur["key"]
                        cs = slice(cur["col0"], cur["col0"] + cur["used"])
                        nc.sync.dma_start(P_hs[fh][fcb, :flo, cs],
                                          cur["lo"][:, : cur["used"]])
                        if fhi:
                            nc.sync.dma_start(P_hs[fh][fcb, flo:flo + fhi, cs],
                                              cur["hi"][:, : cur["used"]])
                        cur["key"] = None
                        cur["used"] = 0

                    def do_window(w, pblk):
                        kw = int(K[w])
                        wid = int(WWID[w])
                        cb, h = int(WCB[w]), int(WH[w])
                        pcol = int(WSTART[w]) - cb * N_PAD - h * HCOL
                        ps = aps.tile([flo, WIN], F32, tag="ps")
                        ps_hi = None
                        if fhi:
                            ps_hi = ahs.tile([fhi, WIN], F32, tag="psh")
                        for j in range(kw):
                            b = pblk + j
                            gt = gtiles[b // GBLK]
                            ch = b % GBLK
                            st = stp.tile([128, WIN], BF16, tag="st")
                            nc.vector.tensor_scalar(
                                st[:, :wid], iota_sb[:, :wid],
                                rel_sb[:, b:b + 1],
                                sval_sb[:, b:b + 1], EQ, MUL)
                            nc.tensor.matmul(
                                ps[:, :wid], gt[:, ch, :flo], st[:, :wid],
                                start=(j == 0), stop=(j == kw - 1))
                            if fhi:
                                nc.tensor.matmul(
                                    ps_hi[:, :wid], gt[:, ch, flo:flo + fhi],
                                    st[:, :wid],
                                    start=(j == 0), stop=(j == kw - 1))
                        # stage into the column-accumulating write group
                        if (cur["key"] != (cb, h)
                                or cur["used"] + wid > STGC):
                            flush()
                        if cur["key"] is None:
                            cur["key"] = (cb, h)
                            cur["col0"] = pcol
                            cur["lo"] = wgp.tile([flo, STGC], FP8,
                                                 name="stg_lo", tag="sl")
                            if fhi:
                                cur["hi"] = wgp.tile([fhi, STGC], FP8,
                                                     name="stg_hi", tag="sh")
                        u = cur["used"]
                        nc.scalar.activation(cur["lo"][:, u:u + wid],
                                             ps[:, :wid], CPY)
                        if fhi:
                            nc.vector.tensor_scalar(
                                cur["hi"][:, u:u + wid], ps_hi[:, :wid],
                                1.0, None, MUL)
                        cur["used"] = u + wid

                    for g in range(ngath + 1):
                        if g < ngath:
                            nb = min(GBLK, NBLK - g * GBLK)
                            gt = gbp.tile([128, GBLK, ELEM_in], BF16, tag="gb")
                            c0 = g * GBLK * 8
                            nc.gpsimd.dma_gather(
                                gt[:, :nb, :], B_in[:, :],
                                idx_sb[:, c0:c0 + nb * 8],
                                nb * 128, nb * 128, ELEM_in)
                            gtiles[g] = gt
                            issued += nb
                        if rs0_due is not None and g >= rs0_due:
                            reduce_scatter(P_hs[0], A_hs[0])
                            inj_from = g + inj_lag
                            rs0_due = None
                        if (inj_from is not None and g >= inj_from
                                and inj_i < len(inj)):
                            inj[inj_i]()
                            inj_i += 1
                        while (w < NWG
                               and pblk + int(K[w]) <= issued):
                            do_window(w, pblk)
                            pblk += int(K[w])
                            w += 1
                            if w == W_HALF:
                                flush()
                                rs0_due = g + RS_DELAY
                    if rs0_due is not None:
                        reduce_scatter(P_hs[0], A_hs[0])
                    flush()
                    reduce_scatter(P_hs[1], A_hs[1])
                    while inj_i < len(inj):
                        inj[inj_i]()
                        inj_i += 1

            # persistent f-major chains + slabs (alive across layers: the
            # f-major pass of layer L runs inside layer L+1's window phase)
            h1T = chainp.tile([128, N_PAD], BF16, name="h1T", tag="chain")
            h2a = chainp.tile([128, N_PAD], BF16, name="h2a", tag="chain")
            h2b = chainp.tile([128, N_PAD], BF16, name="h2b", tag="chain")
            nhi = F2 - 128
            slab1 = slabp.tile([128, N_PAD], BF16, tag="slab", name="slab1")
            slab2 = slabp.tile([128, N_PAD], BF16, tag="slab", name="slab2")
            slab_lo = slabp.tile([128, N_PAD], BF16, tag="slab",
                                 name="slab3a")
            slab_hi = slabp.tile([128, N_PAD], BF16, tag="slab",
                                 name="slab3b")
            d2_sb = dbcp.tile([128, N_PAD], BF16, name="d2", tag="d2t")
            nc.sync.dma_start(d2_sb[:], d2bc_in[:, :])
            nc.vector.memset(slab1[:], 1.0)
            nc.vector.memset(slab2[:], 1.0)

            # ---- L1 ----
            def l1_half(h):
                hc0 = h * HCOL
                a8 = {}
                th = []

                def pre():
                    a8["t"] = auxp.tile([128, HCOL], FP8, tag="a8lo", name="a81")
                    gated_dma(a8["t"][:F1, :], A1h[h][:, :], h)
                th.append(pre)
                for t0 in range(h * HTIL, (h + 1) * HTIL, TG):
                    t1 = min(t0 + TG, (h + 1) * HTIL)

                    def grp(t0=t0, t1=t1):
                        cl = slice(t0 * 128, t1 * 128)
                        ll = slice(t0 * 128 - hc0, t1 * 128 - hc0)
                        nc.vector.tensor_scalar(slab1[:F1, cl],
                                                a8["t"][:F1, ll], 1.0, None,
                                                MUL)
                        nc.vector.tensor_tensor(slab1[:F1, cl],
                                                slab1[:F1, cl], zT[:F1, cl],
                                                ADD)
                        # fm1: h1T = d2 * relu(slab1 + b1) (L2 diagonal)
                        nc.scalar.activation(h1T[:F1, cl], slab1[:F1, cl],
                                             REL, bias=b1_sb[:, 0:1])
                        nc.vector.tensor_tensor(h1T[:F1, cl], h1T[:F1, cl],
                                                d2_sb[:F1, cl], MUL)
                        nm_group(tview(B1, E1), E1,
                                 [(slab1, F1 + 1, w1e_sb[:], 0, F1)],
                                 True, True, t0, t1, nsE1)
                    th.append(grp)
                return th

            if SL >= 2:
                conv_agg(1, B0, E1, F1, 0, P1h, A1h,
                         inject=(l1_half(0) if SL >= 4 else None),
                         inj_lag=999)
            if SL >= 4:
                for t in l1_half(1):
                    t()

            # ---- L2 ----
            def fm2_chunks(h):
                """f-major h2 = relu(W2e^T slab2), pre-scaled by dinv^2
                (only consumer is L3's diagonal)."""
                th = []
                for c0 in range(h * HCOL, (h + 1) * HCOL, CHUNK):
                    def fmc(c0=c0):
                        c1 = min(c0 + CHUNK, (h + 1) * HCOL)
                        cw = c1 - c0
                        tpa = fmpp.tile([128, CHUNK], F32, tag="tpa")
                        nc.tensor.matmul(tpa[:, :cw], w2_sb[:, 0:128],
                                         slab2[:F1 + 1, c0:c1], start=True,
                                         stop=True)
                        nc.scalar.activation(h2a[:, c0:c1], tpa[:, :cw], REL)
                        nc.vector.tensor_tensor(h2a[:, c0:c1], h2a[:, c0:c1],
                                                d2_sb[:, c0:c1], MUL)
                        tpb = fmpp.tile([nhi, CHUNK], F32, tag="tpb")
                        nc.tensor.matmul(tpb[:, :cw], w2_sb[:, 128:F2],
                                         slab2[:F1 + 1, c0:c1], start=True,
                                         stop=True)
                        nc.scalar.activation(h2b[:nhi, c0:c1], tpb[:, :cw],
                                             REL)
                        nc.vector.tensor_tensor(h2b[:nhi, c0:c1],
                                                h2b[:nhi, c0:c1],
                                                d2_sb[:nhi, c0:c1], MUL)
                    th.append(fmc)
                return th

            def l2_half(h):
                hc0 = h * HCOL
                a8 = {}
                th = []

                def pre():
                    a8["t"] = auxp.tile([128, HCOL], FP8, tag="a8lo", name="a82")
                    gated_dma(a8["t"][:F1, :], A2h[h][:, :], h)
                th.append(pre)
                for t0 in range(h * HTIL, (h + 1) * HTIL, TG):
                    t1 = min(t0 + TG, (h + 1) * HTIL)

                    def grp(t0=t0, t1=t1):
                        cl = slice(t0 * 128, t1 * 128)
                        ll = slice(t0 * 128 - hc0, t1 * 128 - hc0)
                        nc.vector.tensor_scalar(slab2[:F1, cl],
                                                a8["t"][:F1, ll], 1.0, None,
                                                MUL)
                        nc.vector.tensor_tensor(slab2[:F1, cl],
                                                slab2[:F1, cl], h1T[:F1, cl],
                                                ADD)
                        nm_group(tview(B2, E2), E2,
                                 [(slab2, F1 + 1, w2_sb[:], 0, F2)],
                                 True, True, t0, t1, nsE2)
                    th.append(grp)
                return th

            def pre3():
                nc.vector.memset(slab_hi[:], 1.0)

            if SL >= 5:
                inj2 = ((l2_half(0) + fm2_chunks(0))
                        if SL >= 6 else None)
                conv_agg(2, B1, E1, F1, 0, P2h, A2h, inject=inj2,
                         inj_lag=999)
            if SL >= 6:
                l2h1 = l2_half(1)
                fm2h1 = fm2_chunks(1)
                l2h1[0]()
                fi = 0
                for k, t in enumerate(l2h1[1:]):
                    t()
                    # fm2 chunk c is ready once the grp covering its columns
                    # ran; interleave to keep PE/Act busy during the chain
                    cols_done = HCOL + min((k + 1) * TG, HTIL) * 128
                    while (fi < len(fm2h1)
                           and HCOL + (fi + 1) * CHUNK <= cols_done):
                        fm2h1[fi]()
                        fi += 1
                while fi < len(fm2h1):
                    fm2h1[fi]()
                    fi += 1

            # ---- L3 ----
            def l3_half(h):
                hc0 = h * HCOL
                a8 = {}
                th = []

                def pre():
                    a8["l"] = auxp.tile([128, HCOL], FP8, tag="a8lo",
                                        name="a83l")
                    gated_dma(a8["l"][:], A3h[h][0:128, :], h)
                    a8["h"] = auxp.tile([nhi, HCOL], FP8, tag="a8hi",
                                        name="a83h")
                    gated_dma(a8["h"][:], A3h[h][128:F2, :], h)
                th.append(pre)
                for t0 in range(h * HTIL, (h + 1) * HTIL, TG):
                    t1 = min(t0 + TG, (h + 1) * HTIL)

                    def grp(t0=t0, t1=t1):
                        cl = slice(t0 * 128, t1 * 128)
                        ll = slice(t0 * 128 - hc0, t1 * 128 - hc0)
                        nc.vector.tensor_scalar(slab_lo[:, cl],
                                                a8["l"][:, ll], 1.0, None,
                                                MUL)
                        nc.vector.tensor_tensor(slab_lo[:, cl],
                                                slab_lo[:, cl], h2a[:, cl],
                                                ADD)
                        nc.vector.tensor_scalar(slab_hi[:nhi, cl],
                                                a8["h"][:, ll], 1.0, None,
                                                MUL)
                        nc.vector.tensor_tensor(slab_hi[:nhi, cl],
                                                slab_hi[:nhi, cl],
                                                h2b[:nhi, cl], ADD)
                        nm_group(h3m_slice, EP,
                                 [(slab_lo, 128, w3a_sb[:], 0, F3),
                                  (slab_hi, nhi + 1, w3b_sb[:], 0, F3)],
                                 True, False, t0, t1, nsEP)
                    th.append(grp)
                return th

            if SL >= 7:
                conv_agg(3, B2, E2, 128, F2 - 128, P3h, A3h,
                         inject=([pre3] + l3_half(0) if SL >= 8 else None),
                         inj_lag=999, gate_lag=13)
            # ---- pooling + MLP interleaved with the h1 transform ------
            # bins over half-0 graphs (and MLP part a) are emitted BEFORE the
            # h1 transform drain: their data is ready mid-phase, so they fill
            # the RS-h1 collective window on PE/DVE/DMA; bins over half-1
            # graphs + MLP part b close out the tail.
            FCH = [(0, 128), (128, 256), (256, F3)]
            gT = holdp.tile([128, 3 * GPC], BF16, name="gT")
            PGMAXT = meta["PGMAXT"]
            PAD_B, NIDX_B = meta["PAD_B"], meta["NIDX_B"]
            OFF_B, NBIN, GBIN = meta["OFF_B"], meta["NBIN"], meta["GBIN"]
            HB = NBIN // 2
            with (
                tc.tile_pool(name="poolg", bufs=3) as pgp,
                tc.tile_pool(name="poolps", bufs=2, space="PSUM") as pps,
                tc.tile_pool(name="mlpw", bufs=1) as mwp,
                tc.tile_pool(name="mlps", bufs=1) as msp,
            ):
                def pool_bin(b):
                    n = int(NIDX_B[b])
                    nt = n // 128
                    pad = int(PAD_B[b])
                    cap = int(PGMAXT[b])
                    o16 = int(OFF_B[b]) // 16
                    pt = pgp.tile([128, 6, EP], BF16, tag="pg")
                    nc.gpsimd.dma_gather(
                        pt[:, :nt, :], h3t[0:cap, :],
                        slot_sb[:, o16:o16 + n // 16],
                        n, n, EP)
                    gcol = b * GBIN
                    for i, (f0, f1) in enumerate(FCH):
                        csz = f1 - f0
                        tps = pps.tile([128, 6 * 128], BF16,
                                       tag=f"tps{i}", name=f"tps{i}")
                        for t in range(nt):
                            nc.tensor.transpose(
                                tps[:csz, t * 128:(t + 1) * 128],
                                pt[:, t, i * 128:i * 128 + csz],
                                ident[:])
                        nc.vector.tensor_reduce(
                            gT[:csz, i * GPC + gcol: i * GPC + gcol + GBIN],
                            tps[:csz, :GBIN * pad].rearrange(
                                "p (g s) -> p g s", s=pad),
                            mybir.AxisListType.X, MAX)

                ksz = [128, 128, F3 - 256]
                wg1_sb = [mwp.tile([ksz[i], D1], BF16, name=f"wg1_{i}",
                                   tag=f"wg1_{i}") for i in range(3)]
                for i in range(3):
                    r0 = 128 * i
                    nc.sync.dma_start(wg1_sb[i][:], Wg1_in[r0:r0 + ksz[i], :])
                bg1_sb = msp.tile([128, D1 // 128], BF16)
                nc.sync.dma_start(bg1_sb[:], bg1_in[:, :])
                wg2_sb = [mwp.tile([128, D2], BF16, name=f"wg2_{i}",
                                   tag=f"wg2_{i}") for i in range(D1 // 128)]
                for i in range(D1 // 128):
                    nc.sync.dma_start(wg2_sb[i][:],
                                      Wg2_in[i * 128:(i + 1) * 128, :])
                bg2_sb = msp.tile([128, 1], F32)
                nc.sync.dma_start(bg2_sb[:], bg2_in[:, :])
                wf1_sb = mwp.tile([128, D3], BF16)
                nc.sync.dma_start(wf1_sb[:], Wf1_in[:, :])
                bf1_sb = msp.tile([128, D3 // 128], BF16)
                nc.sync.dma_start(bf1_sb[:], bf1_in[:, :])
                wf2_sb = [mwp.tile([128, D4], BF16, name=f"wf2_{i}",
                                   tag=f"wf2_{i}") for i in range(D3 // 128)]
                for i in range(D3 // 128):
                    nc.sync.dma_start(wf2_sb[i][:],
                                      Wf2_in[i * 128:(i + 1) * 128, :])
                bf2_sb = msp.tile([128, D4 // 128], BF16)
                nc.sync.dma_start(bf2_sb[:], bf2_in[:, :])
                wo_sb = [mwp.tile([128, 1], BF16, name=f"wo_{i}",
                                  tag=f"wo_{i}") for i in range(D4 // 128)]
                for i in range(D4 // 128):
                    nc.sync.dma_start(wo_sb[i][:],
                                      Wo_in[i * 128:(i + 1) * 128, :])
                bo_sb = msp.tile([1, 1], BF16)
                nc.sync.dma_start(bo_sb[:], bo_in[:, :])

                def mlp_half(g0, g1w, sfx):
                    gw = g1w - g0
                    g1t = msp.tile([128, (D1 // 128) * gw], BF16,
                                   name=f"g1{sfx}", tag=f"g1{sfx}")
                    for m in range(D1 // 128):
                        ps = nmpp.tile([128, F3], F32, tag="ps",
                                       name="ps")[:, :gw]
                        for i in range(3):
                            nc.tensor.matmul(
                                ps[:], wg1_sb[i][:, m * 128:(m + 1) * 128],
                                gT[:ksz[i], i * GPC + g0: i * GPC + g1w],
                                start=(i == 0), stop=(i == 2))
                        nc.scalar.activation(g1t[:, m * gw:(m + 1) * gw],
                                             ps[:], REL,
                                             bias=bg1_sb[:, m:m + 1])
                    ps = nmpp.tile([128, F3], F32, tag="ps",
                                   name="ps")[:, :gw]
                    for i in range(D1 // 128):
                        nc.tensor.matmul(ps[:], wg2_sb[i][:],
                                         g1t[:, i * gw:(i + 1) * gw],
                                         start=(i == 0),
                                         stop=(i == D1 // 128 - 1))
                    g2 = msp.tile([128, gw], BF16, name=f"g2{sfx}",
                                  tag=f"g2{sfx}")
                    nc.vector.tensor_scalar(g2[:], ps[:], bg2_sb[:, 0:1],
                                            None, ADD)
                    c1t = msp.tile([128, (D3 // 128) * gw], BF16,
                                   name=f"c1{sfx}", tag=f"c1{sfx}")
                    for m in range(D3 // 128):
                        ps = nmpp.tile([128, F3], F32, tag="ps",
                                       name="ps")[:, :gw]
                        nc.tensor.matmul(
                            ps[:], wf1_sb[:, m * 128:(m + 1) * 128],
                            g2[:], start=True, stop=True)
                        nc.scalar.activation(c1t[:, m * gw:(m + 1) * gw],
                                             ps[:], REL,
                                             bias=bf1_sb[:, m:m + 1])
                    c2t = msp.tile([128, (D4 // 128) * gw], BF16,
                                   name=f"c2{sfx}", tag=f"c2{sfx}")
                    for m in range(D4 // 128):
                        ps = nmpp.tile([128, F3], F32, tag="ps",
                                       name="ps")[:, :gw]
                        for i in range(D3 // 128):
                            nc.tensor.matmul(
                                ps[:], wf2_sb[i][:, m * 128:(m + 1) * 128],
                                c1t[:, i * gw:(i + 1) * gw],
                                start=(i == 0), stop=(i == D3 // 128 - 1))
                        nc.scalar.activation(c2t[:, m * gw:(m + 1) * gw],
                                             ps[:], REL,
                                             bias=bf2_sb[:, m:m + 1])
                    pso = nmpp.tile([128, F3], F32, tag="ps",
                                    name="pso")[0:1, :gw]
                    for i in range(D4 // 128):
                        nc.tensor.matmul(pso[:], wo_sb[i][:],
                                         c2t[:, i * gw:(i + 1) * gw],
                                         start=(i == 0),
                                         stop=(i == D4 // 128 - 1))
                    o_sb = msp.tile([1, gw], F32, name=f"o{sfx}",
                                    tag=f"o{sfx}")
                    nc.scalar.activation(o_sb[:], pso[:], SIG,
                                         bias=bo_sb[:, 0:1])
                    nc.sync.dma_start(out_d[:, g0:g1w], o_sb[:])

                if SL >= 9:
                    for b in sorted(range(HB), key=lambda b: PGMAXT[b]):
                        pool_bin(b)
                    if SL >= 10:
                        mlp_half(0, GPC // 2, "a")
                if SL >= 8:
                    for t in l3_half(1):
                        t()
                if SL >= 9:
                    for b in sorted(range(HB, NBIN), key=lambda b: PGMAXT[b]):
                        pool_bin(b)
                    if SL >= 10:
                        mlp_half(GPC // 2, GPC, "b")
            if SL < 10:
                with tc.tile_pool(name="zo", bufs=1) as zop:
                    zo = zop.tile([1, GPC], F32)
                    nc.vector.memset(zo[:], 0.0)
                    nc.sync.dma_start(out_d[:, :], zo[:])


    nc.compile()
    return nc


LAST_EXEC_NS = None
LAST_RES = None


def kernel(**inputs):
    global LAST_EXEC_NS, LAST_RES
    x = np.asarray(inputs["x"])
    edge_index = np.asarray(inputs["edge_index"])
    batch = np.asarray(inputs["batch"])
    weights = {k: np.asarray(v) for k, v in inputs.items()
               if k not in ("x", "edge_index", "batch")}
    meta, in_maps = _plan(x, edge_index, batch, weights)
    nc = _build(meta)
    res = run_bass_kernel_spmd(nc, in_maps, core_ids=list(range(NC)))
    LAST_RES = res
    LAST_EXEC_NS = res.exec_time_ns
    out_sorted = np.concatenate(
        [np.asarray(res.results[c]["out_d"][0], np.float32)
         for c in range(NC)])
    out = np.empty_like(out_sorted)
    out[meta["UNPERM"]] = out_sorted
    return out.reshape(-1, 1)
